# revision 36
# baseline (speedup 1.0000x reference)
"""AttentionBottleNeck Trainium2 kernel — 8-core data-parallel over batch.

Math (per batch, x [C=256, L=4096]):
  LayerNorm over C -> grouped 1x1 conv logits -> softmax over L
  -> V = val 1x1 conv -> A = softmax-weighted pool of V -> final linear.

Device per batch (transposed-domain design):
  xa   [c=128, 2, L]        natural bf16 (host pre-converts)
  xt3  [l=128, 3, 32, 128]  slabs 0-1: host-pre-transposed x; slab 2 col 0
                            holds rs so the pooling matmul also yields sumE
  sqs  [l=128, 32] = sum_c x^2  (DVE square + bf16 tree + reduce)
  lnv = Ln(sqs/256+eps); s = exp(-lnv/2); rs = exp(+lnv/2)   [ACT tiny]
  lgp [hq,512]x8 = aw''T @ xa (PE) -> bf16 evac (ACT) -> XBAR halves ->
  lgT [l, 32, 128]; gp = lgT*s + ln s (DVE/GPS); gT = exp(gp) (ACT)
  pool: out[hq, 384] += gT_k.T @ xt3[:, :, k, :]  — cols 0:256 = A-unnorm,
        col 256 = sumE (rs slot), cols 257+ ignored
  device returns [PB, 128, 257] (pooled block + sumE column)
Two batch-streams are interleaved (generator round-robin, staggered) so the
strict-FIFO engine queues always hold ready work from the other batch.
Host: A = out[:,:256]/out[:,256], gamma folded into aw'' (zero-sum cols kill
mu), val conv applied after pooling (commutes), head strips, final linear.
mu^2 in var is dropped (relative var error ~0.4%).
"""
import os
import sys
import numpy as np

sys.path.insert(0, "/opt/trn_rl_repo")

B, C, H, W = 64, 256, 64, 64
HEADS, Q, FH = 8, 16, 512
L = H * W            # 4096
EPS = 1e-6
NCORES = 8
PB = B // NCORES     # 8 batches per core
NT = 32              # 128-wide l-chunks

_CACHE = {}
LAST_RESULTS = None


def _patch_act_tables():
    """Make every act func resolve to natural_log_exp_and_others (has exp,
    ln AND square) -> one table load total instead of ln/exp thrash."""
    from concourse import bacc, hw_specs

    if getattr(bacc, "_act_tables_patched", False):
        return
    orig = hw_specs.get_activation_tables

    def patched(arch):
        tabs = dict(orig(arch))
        pref = "natural_log_exp_and_others"
        if pref not in tabs:
            return tabs
        pset = tabs[pref]
        return {k: (v if k == pref else v - pset) for k, v in tabs.items()}

    bacc.get_activation_tables = patched
    bacc._act_tables_patched = True


def _build_nc():
    import concourse.bass as bass  # noqa: F401
    import concourse.tile as tile
    from concourse import bacc, mybir
    from contextlib import ExitStack

    _patch_act_tables()

    f32 = mybir.dt.float32
    bf16 = mybir.dt.bfloat16
    Alu = mybir.AluOpType
    Act = mybir.ActivationFunctionType

    nc = bacc.Bacc("TRN2", target_bir_lowering=False, debug=False, num_devices=NCORES)

    x_in = nc.dram_tensor("x", [PB, 128, 2, L], bf16, kind="ExternalInput").ap()
    xt_in = nc.dram_tensor("xt", [PB, 128, 2, NT, 128], bf16,
                           kind="ExternalInput").ap()
    aw_in = nc.dram_tensor("aw", [128, 2, 128], bf16, kind="ExternalInput").ap()
    out_d = nc.dram_tensor("acore", [PB, 128, 257], f32, kind="ExternalOutput").ap()

    with tile.TileContext(nc) as tc, ExitStack() as ctx:
        P = lambda **kw: ctx.enter_context(tc.tile_pool(**kw))
        wpool = P(name="w", bufs=1)
        xpool = P(name="x", bufs=2)
        tpool = P(name="t", bufs=2)
        lpool = P(name="l", bufs=2)
        gpool = P(name="g", bufs=2)
        spool = P(name="s", bufs=2)
        opool = P(name="o", bufs=2)
        ps_lg = P(name="pslg", bufs=4, space="PSUM")
        ps_a = P(name="psa", bufs=3, space="PSUM")

        awT = wpool.tile([128, 2, 128], bf16, tag="awT")
        nc.sync.dma_start(out=awT[:], in_=aw_in[:])
        eps_sb = wpool.tile([128, 1], f32, tag="eps")
        zero_sb = wpool.tile([128, 1], f32, tag="zero")
        nc.vector.memset(eps_sb[:], EPS)
        nc.vector.memset(zero_sb[:], 0.0)

        def body(pb):
            """Per-batch pipeline as a generator; yields between instruction
            groups so two batches can interleave in the engine FIFOs."""
            # loads on the scalar hwdge ring; sync ring reserved for XBAR
            # (the XBAR block corrupts data when driven from two rings).
            xa = xpool.tile([128, 2, L], bf16, tag="xa")
            nc.scalar.dma_start(out=xa[:], in_=x_in[pb])
            xt3 = tpool.tile([128, 3, NT, 128], bf16, tag="xt3")
            nc.scalar.dma_start(out=xt3[:, 0:2], in_=xt_in[pb])
            yield

            # sum_c x^2 per l: square halves + bf16 tree, 16 chunks at a time
            sqs = spool.tile([128, NT], f32, tag="sqs")
            sqa = spool.tile([128, 16, 128], bf16, tag="sqa")
            sqb = spool.tile([128, 16, 128], bf16, tag="sqb")
            for g in range(2):
                ks = slice(g * 16, (g + 1) * 16)
                nc.vector.tensor_mul(sqa[:], xt3[:, 0, ks, :], xt3[:, 0, ks, :])
                nc.vector.tensor_mul(sqb[:], xt3[:, 1, ks, :], xt3[:, 1, ks, :])
                yield
                nc.vector.tensor_add(sqa[:], sqa[:], sqb[:])
                nc.vector.tensor_add(sqb[:, :, 0:64], sqa[:, :, 0:64],
                                     sqa[:, :, 64:128])
                nc.vector.tensor_add(sqa[:, :, 0:32], sqb[:, :, 0:32],
                                     sqb[:, :, 32:64])
                nc.vector.tensor_add(sqb[:, :, 0:16], sqa[:, :, 0:16],
                                     sqa[:, :, 16:32])
                nc.vector.tensor_reduce(sqs[:, ks], sqb[:, :, 0:16],
                                        mybir.AxisListType.X, Alu.add)
                yield

            # stats: lnv = ln(sqs/256+eps); s = exp(-.5lnv); rs -> xt3 slab 2
            lnv = spool.tile([128, NT], f32, tag="lnv")
            s_t = spool.tile([128, NT], f32, tag="s_t")
            lns = spool.tile([128, NT], f32, tag="lns")
            nc.scalar.activation(lnv[:], sqs[:], Act.Ln, bias=eps_sb[:],
                                 scale=1.0 / 256.0)
            nc.scalar.activation(s_t[:], lnv[:], Act.Exp, bias=zero_sb[:],
                                 scale=-0.5)
            nc.scalar.activation(xt3[:, 2, :, 0:1], lnv[:, :, None], Act.Exp,
                                 bias=zero_sb[:], scale=0.5)
            nc.vector.tensor_scalar_mul(lns[:], lnv[:], -0.5)
            yield

            # logits natural -> bf16, then per-half XBAR + scale + exp
            lgn = lpool.tile([128, L], bf16, tag="lgn")
            lgT = gpool.tile([128, NT, 128], bf16, tag="lgT")
            gp = gpool.tile([128, NT, 128], bf16, tag="gp")
            gT = gpool.tile([128, NT, 128], bf16, tag="gT")
            ap = ps_a.tile([128, 384], f32, tag="ap")
            for g in range(2):
                for ch in range(g * 4, g * 4 + 4):
                    lgp = ps_lg.tile([128, 512], f32, tag="lgp")
                    for h in range(2):
                        nc.tensor.matmul(lgp[:], awT[:, h, :],
                                         xa[:, h, ch * 512:(ch + 1) * 512],
                                         start=(h == 0), stop=(h == 1))
                    nc.scalar.activation(lgn[:, ch * 512:(ch + 1) * 512],
                                         lgp[:], Act.Copy, bias=0.0)
                    yield
                ks = slice(g * 16, (g + 1) * 16)
                nc.sync.dma_start(out=lgT[:, ks, :],
                                  in_=lgn[:, g * 2048:(g + 1) * 2048],
                                  transpose=True)
                yield
                for q in range(2):
                    for k in range(g * 16 + q * 8, g * 16 + q * 8 + 8):
                        eng = nc.gpsimd if (k % 4 == 3) else nc.vector
                        eng.tensor_scalar(gp[:, k, :], lgT[:, k, :],
                                          s_t[:, k:k + 1], lns[:, k:k + 1],
                                          Alu.mult, Alu.add)
                    yield
                    qs = slice(g * 16 + q * 8, g * 16 + q * 8 + 8)
                    nc.scalar.activation(gT[:, qs, :], gp[:, qs, :], Act.Exp,
                                         bias=zero_sb[:])
                    yield
                    # pool: [hq, 384] += gT_k.T @ xt3[:, :, k, :]
                    # cols 0:256 = A-unnorm, col 256 = sumE, 257+ junk
                    for k in range(g * 16 + q * 8, g * 16 + q * 8 + 8):
                        nc.tensor.matmul(ap[:], gT[:, k, :], xt3[:, :, k, :],
                                         start=(k == 0), stop=(k == NT - 1))
                    yield

            # evac pooled block + sumE to SBUF and store; host normalizes
            a_sb = opool.tile([128, 257], f32, tag="a_sb")
            nc.scalar.activation(a_sb[:], ap[:, 0:257], Act.Copy, bias=0.0)
            nc.scalar.dma_start(out=out_d[pb], in_=a_sb[:])
            yield

        # drive two batch-streams interleaved to fill the engine FIFOs;
        # stagger the first stream half a body ahead so the pair never
        # runs in lockstep (lockstep = bubbles at pair boundaries)
        from collections import deque
        g0 = body(0)
        for _ in range(8):
            next(g0)
        streams = deque([g0, body(1)])
        next_pb = 2
        while streams:
            g = streams.popleft()
            try:
                next(g)
                streams.append(g)
            except StopIteration:
                if next_pb < PB:
                    streams.append(body(next_pb))
                    next_pb += 1

    nc.compile()
    return nc


def _get_nc():
    if "nc" not in _CACHE:
        _CACHE["nc"] = _build_nc()
    return _CACHE["nc"]


def _host_fold(ln_gamma, ln_beta, attn_w, val_w, val_b):
    g = np.asarray(ln_gamma, np.float64)
    aw = np.asarray(attn_w, np.float64)          # [h, q, c/h]
    Wb = np.zeros((256, 128))
    for h in range(HEADS):
        Wb[32 * h:32 * h + 32, 16 * h:16 * h + 16] = \
            (aw[h] * g[32 * h:32 * h + 32][None, :]).T
    Wb -= Wb.mean(axis=0, keepdims=True)         # zero-sum cols -> mu drops out
    vw = np.asarray(val_w, np.float64) * g[None, :]
    vw2 = vw - vw.mean(axis=1, keepdims=True)    # zero-sum rows -> mu drops out
    c_v = np.asarray(val_w, np.float64) @ np.asarray(ln_beta, np.float64) \
        + np.asarray(val_b, np.float64)
    return Wb, vw2, c_v


def kernel(x, ln_gamma, ln_beta, attn_w, val_w, val_b, fin_w, fin_b):
    global LAST_RESULTS
    from concourse.bass_utils import run_bass_kernel_spmd
    import ml_dtypes

    nc = _get_nc()
    Wb, vw2, c_v = _host_fold(ln_gamma, ln_beta, attn_w, val_w, val_b)
    bf = ml_dtypes.bfloat16
    awT = np.ascontiguousarray(
        Wb.reshape(2, 128, 128).transpose(1, 0, 2)).astype(bf)
    # x: [B, 256, 64, 64] -> [B, c-in-half(128), half(2), L] bf16
    xb = np.asarray(x, np.float32).reshape(B, 2, 128, L)   # [B, h, cc, l]
    xr = np.ascontiguousarray(xb.transpose(0, 2, 1, 3)).astype(bf)
    # host-side transpose: xt[b, p, h, k, cc] = x[b, h, cc, k*128+p]
    xt = np.ascontiguousarray(
        xb.reshape(B, 2, 128, NT, 128).transpose(0, 4, 1, 3, 2)).astype(bf)
    in_maps = [
        {"x": xr[PB * i:PB * (i + 1)], "xt": xt[PB * i:PB * (i + 1)],
         "aw": awT}
        for i in range(NCORES)
    ]
    res = run_bass_kernel_spmd(
        nc, in_maps, list(range(NCORES)),
        trace=bool(int(os.environ.get("KTRACE", "0"))))
    LAST_RESULTS = res
    A_raw = np.concatenate([r["acore"] for r in res.results], 0)  # [64,128,257]
    A_dev = A_raw[:, :, 0:256] / A_raw[:, :, 256:257]

    # host epilogue: val-conv after pooling, head strips, final linear
    A_fin = A_dev.astype(np.float64) @ vw2.T + c_v[None, None, :]  # [64,128,256]
    rows = np.arange(128)
    cols = 32 * (rows // 16)[:, None] + np.arange(32)[None, :]
    A_strip = A_fin[:, rows[:, None], cols]                        # [64,128,32]
    Aflat = A_strip.reshape(B, Q * C)
    out = Aflat @ np.asarray(fin_w, np.float64).T + np.asarray(fin_b, np.float64)
    return out.astype(np.float32)
